# revision 15
# baseline (speedup 1.0000x reference)
"""DiagonalLinear: y = x * w + b (elementwise over features).

x: (16384, 4096) f32, w/b: (4096,) f32.
Sharding: data-parallel over batch across 8 NeuronCores (2048 rows each),
weight/bias replicated.

Per core: HWDGE loads x row-tiles [128, 4096] into `tin` (SP ring), mul+add
into `tout` against partition-broadcast const tiles, HWDGE stores `tout`
(ACT ring). The 64 MiB of load+store traffic saturates the 16-SDMA fabric
(~435 GB/s), so everything else stays off it:
  - w/b are broadcast on-chip by GpSimd partition_broadcast after one 32 KiB
    DMA (not a 4 MiB DMA partition-broadcast),
  - the elementwise work is split DVE (13 tiles) / GpSimd (3 tiles) so the
    vector engine (~8.8us/tile) stops being the pacer.
Separate in/out pools keep every tile at two actors so most instructions
carry a single sync-wait; bacc's generate_event_semaphores splits the rest.
"""

import numpy as np

import concourse.bacc as bacc
import concourse.bass as bass
import concourse.mybir as mybir
import concourse.tile as tile
from concourse.bass_utils import run_bass_kernel_spmd

N_CORES = 8
BATCH = 16384
D = 4096
ROWS_PER_CORE = BATCH // N_CORES  # 2048
P = 128

# Tunables
BUFS_IN = 5    # load runway: 5 x 2MiB covers the ~24us const-setup latency
BUFS_OUT = 4
GP_TILES = frozenset({3, 8, 13})  # tiles computed on GpSimd instead of DVE

_CACHE = {}


def build_nc(bufs_in=BUFS_IN, bufs_out=BUFS_OUT, gp_tiles=GP_TILES):
    nc = bacc.Bacc()
    f32 = mybir.dt.float32
    x = nc.dram_tensor("x", [ROWS_PER_CORE, D], f32, kind="ExternalInput")
    wb_in = nc.dram_tensor("wb", [1, 2 * D], f32, kind="ExternalInput")
    y = nc.dram_tensor("y", [ROWS_PER_CORE, D], f32, kind="ExternalOutput")

    n_tiles = ROWS_PER_CORE // P  # 16
    x_r = x.rearrange("(n p) d -> n p d", p=P)
    y_r = y.rearrange("(n p) d -> n p d", p=P)

    with tile.TileContext(nc) as tc:
        with (
            tc.tile_pool(name="consts", bufs=1) as cpool,
            tc.tile_pool(name="tin", bufs=bufs_in) as pin,
            tc.tile_pool(name="tout", bufs=bufs_out) as pout,
        ):
            consts = cpool.tile([P, 2 * D], f32)  # [:, :D]=w, [:, D:]=b
            scratch = cpool.tile([P, 1], f32)
            with tc.high_priority():
                # 32 KiB DMA into partition 0 + on-chip GpSimd broadcast keeps
                # the const replication off the (saturated) SDMA fabric. The
                # in-place broadcast rewrites partition 0 with its own values.
                nc.sync.dma_start(consts[0:1, :], wb_in[:, :])
                nc.gpsimd.partition_broadcast(consts[:, 0:D], consts[0:1, 0:D])
                nc.gpsimd.partition_broadcast(consts[:, D : 2 * D], consts[0:1, D : 2 * D])
                # absorb the const deps into DVE's clock
                nc.vector.tensor_copy(scratch[:, :], consts[:, 0:1])
                nc.vector.tensor_copy(scratch[:, :], consts[:, D : D + 1])

            wt = consts[:, 0:D]
            bt = consts[:, D : 2 * D]
            for i in range(n_tiles):
                eng = nc.gpsimd if i in gp_tiles else nc.vector
                tin = pin.tile([P, D], f32)
                tout = pout.tile([P, D], f32)
                nc.sync.dma_start(tin[:, :], x_r[i])
                eng.tensor_mul(tout[:, :], tin[:, :], wt)
                eng.tensor_add(tout[:, :], tout[:, :], bt)
                nc.scalar.dma_start(y_r[i], tout[:, :])
    nc.compile()
    return nc


def _get_nc():
    if "nc" not in _CACHE:
        _CACHE["nc"] = build_nc()
    return _CACHE["nc"]


def run(input, weight, bias, nc=None, **spmd_kwargs):
    if nc is None:
        nc = _get_nc()
    x = np.ascontiguousarray(input, dtype=np.float32)
    wb = np.ascontiguousarray(
        np.stack([np.asarray(weight), np.asarray(bias)]).astype(np.float32)
    ).reshape(1, 2 * D)
    in_maps = [
        {"x": x[c * ROWS_PER_CORE : (c + 1) * ROWS_PER_CORE], "wb": wb}
        for c in range(N_CORES)
    ]
    res = run_bass_kernel_spmd(nc, in_maps, core_ids=list(range(N_CORES)), **spmd_kwargs)
    out = np.concatenate([r["y"] for r in res.results], axis=0)
    return out, res


def kernel(input, weight, bias):
    out, _ = run(input, weight, bias)
    return out


# revision 16
# speedup vs baseline: 1.1206x; 1.1206x over previous
"""DiagonalLinear: y = x * w + b (elementwise over features).

x: (16384, 4096) f32, w/b: (4096,) f32.
Sharding: data-parallel over batch across 8 NeuronCores (2048 rows each),
weight/bias replicated.

Per core: HWDGE loads x row-tiles [128, 4096] into `tin` on the SP ring,
DVE computes mul+add into `tout` against partition-broadcast const tiles,
HWDGE stores `tout` on the ACT ring. HWDGE rings drain FIFO per issuing
engine, so the const-broadcast DMAs go on the ACT ring (idle at start)
to keep the first x-load — and therefore the first DVE op — early.
Separate in/out pools keep every tile at two actors so most instructions
carry a single sync-wait; bacc's generate_event_semaphores splits the rest.
"""

import numpy as np

import concourse.bacc as bacc
import concourse.bass as bass
import concourse.mybir as mybir
import concourse.tile as tile
from concourse.bass_utils import run_bass_kernel_spmd

N_CORES = 8
BATCH = 16384
D = 4096
ROWS_PER_CORE = BATCH // N_CORES  # 2048
P = 128

# Tunables
BUFS_IN = 4
BUFS_OUT = 4

_CACHE = {}


def build_nc(bufs_in=BUFS_IN, bufs_out=BUFS_OUT):
    nc = bacc.Bacc()
    f32 = mybir.dt.float32
    x = nc.dram_tensor("x", [ROWS_PER_CORE, D], f32, kind="ExternalInput")
    wb_in = nc.dram_tensor("wb", [1, 2 * D], f32, kind="ExternalInput")
    y = nc.dram_tensor("y", [ROWS_PER_CORE, D], f32, kind="ExternalOutput")

    n_tiles = ROWS_PER_CORE // P  # 16
    x_r = x.rearrange("(n p) d -> n p d", p=P)
    y_r = y.rearrange("(n p) d -> n p d", p=P)

    with tile.TileContext(nc) as tc:
        with (
            tc.tile_pool(name="consts", bufs=1) as cpool,
            tc.tile_pool(name="tin", bufs=bufs_in) as pin,
            tc.tile_pool(name="tout", bufs=bufs_out) as pout,
        ):
            consts = cpool.tile([P, 2 * D], f32)  # [:, :D]=w, [:, D:]=b
            scratch = cpool.tile([P, 1], f32)
            wt = consts[:, 0:D]
            bt = consts[:, D : 2 * D]
            with tc.high_priority():
                # const broadcasts ride the ACT ring so the SP ring's first
                # x-load isn't queued behind them (rings drain FIFO)
                nc.scalar.dma_start(wt, wb_in[:, 0:D].partition_broadcast(P))
                nc.scalar.dma_start(bt, wb_in[:, D : 2 * D].partition_broadcast(P))
                # absorb the const deps into DVE's clock
                nc.vector.tensor_copy(scratch[:, :], consts[:, 0:1])
                nc.vector.tensor_copy(scratch[:, :], consts[:, D : D + 1])

            for i in range(n_tiles):
                tin = pin.tile([P, D], f32)
                tout = pout.tile([P, D], f32)
                nc.sync.dma_start(tin[:, :], x_r[i])
                nc.vector.tensor_mul(tout[:, :], tin[:, :], wt)
                nc.vector.tensor_add(tout[:, :], tout[:, :], bt)
                nc.scalar.dma_start(y_r[i], tout[:, :])
    nc.compile()
    return nc


def _get_nc():
    if "nc" not in _CACHE:
        _CACHE["nc"] = build_nc()
    return _CACHE["nc"]


def run(input, weight, bias, nc=None, **spmd_kwargs):
    if nc is None:
        nc = _get_nc()
    x = np.ascontiguousarray(input, dtype=np.float32)
    wb = np.ascontiguousarray(
        np.stack([np.asarray(weight), np.asarray(bias)]).astype(np.float32)
    ).reshape(1, 2 * D)
    in_maps = [
        {"x": x[c * ROWS_PER_CORE : (c + 1) * ROWS_PER_CORE], "wb": wb}
        for c in range(N_CORES)
    ]
    res = run_bass_kernel_spmd(nc, in_maps, core_ids=list(range(N_CORES)), **spmd_kwargs)
    out = np.concatenate([r["y"] for r in res.results], axis=0)
    return out, res


def kernel(input, weight, bias):
    out, _ = run(input, weight, bias)
    return out
